# revision 8
# baseline (speedup 1.0000x reference)
"""JGCNConv Bass kernel for 8 trn2 NeuronCores.

Math (per batch b):
    rx   = R_w @ x                                  [RDIM, N]
    th   = -dist[m,n] = 2*rx_m.rx_n - sq[m] - sq[n]
    A1   = exp(th);  deg1 = rowsum(A1);  d1 = rsqrt(max(deg1,1))
    A1n  = d1[m] A1 d1[n] = exp(th + lnd1[m] + lnd1[n])
    deg2 = rowsum(A1n) + rowsum(old_A); d2 = rsqrt(max(deg2,1))
    A2   = d2[m] (A1n + old_A) d2[n]
         = exp(th + lnd1[m]+lnd1[n]+lnd2[m]+lnd2[n]) + d2[m]*old_A*d2[n]
    out  = (W_w @ x) @ A2^T + W_b
Outputs: (out, A2).

Sharding: cores 0..3 -> batch 0, cores 4..7 -> batch 1; each core owns
MLOC = N/4 rows of the NxN adjacency.  Two 4KB AllGathers (deg1, deg2)
exchange full-N degree vectors inside each 4-core group.  The program is
identical on all cores (SPMD): every per-core difference comes from the
input shards; row-scalings use locally computed degrees, column/free-dim
vectors come from the AllGathers.
"""

import os
import sys
from contextlib import ExitStack

import numpy as np

for _p in ("/opt/trn_rl_repo", "/root/.axon_site/_ro/trn_rl_repo"):
    if os.path.isdir(_p) and _p not in sys.path:
        sys.path.insert(0, _p)

import concourse.bass as bass  # noqa: E402
import concourse.mybir as mybir  # noqa: E402
import concourse.tile as tile  # noqa: E402
from concourse import bacc  # noqa: E402
from concourse.masks import make_identity  # noqa: E402

F32 = mybir.dt.float32
AF = mybir.ActivationFunctionType
ALU = mybir.AluOpType

CFG = dict(B=2, FIN=256, FOUT=256, N=4096, RDIM=16, n_cores=8)


def build_kernel_body(ctx, tc, io, cfg):
    nc = tc.nc
    P = 128
    B, FIN, FOUT, N, RDIM = (
        cfg["B"], cfg["FIN"], cfg["FOUT"], cfg["N"], cfg["RDIM"],
    )
    n_cores = cfg["n_cores"]
    GROUP = n_cores // B
    MLOC = N // GROUP           # rows of A owned by this core
    CW = min(512, N)            # column chunk width (one psum bank)
    NCC = N // CW               # column chunks
    NT = MLOC // P              # 128-row tiles in the local block
    KF = FIN // P               # k-chunks over FIN
    OT = FOUT // P              # output row tiles
    KA = RDIM + 2               # augmented contraction dim
    NU = N // P                 # 128-wide n chunks (for zT)
    HW_ = min(512, MLOC)        # free-width for xa matmuls
    NH = MLOC // HW_
    NS = CW // P                # transposed sub-chunks per column chunk

    x_d = io["x"]
    xl_d = io["x_local"]
    oa_d = io["old_A"]
    rw_d = io["R_w"]
    ww_d = io["W_w"]
    wb_d = io["W_b"]
    aout_d = io["A_out"]
    out_d = io["out_f"]
    cc1i, cc1o, cc2i, cc2o = io["cc1i"], io["cc1o"], io["cc2i"], io["cc2o"]

    groups = [list(range(g * GROUP, (g + 1) * GROUP)) for g in range(B)]

    const = ctx.enter_context(tc.tile_pool(name="const", bufs=1))
    pers = ctx.enter_context(tc.tile_pool(name="pers", bufs=1))
    stream = ctx.enter_context(tc.tile_pool(name="stream", bufs=3))

    # ---- constants -------------------------------------------------------
    ident = const.tile([P, P], F32, tag="ident")
    make_identity(nc, ident)
    ones_r = const.tile([RDIM, 1], F32, tag="ones_r")
    nc.vector.memset(ones_r, 1.0)
    ones_c = const.tile([1, P], F32, tag="ones_c")
    nc.vector.memset(ones_c, 1.0)
    ones_dram = nc.inline_tensor(np.ones((1, N), dtype=np.float32), name="ones_dram").ap()
    wb_sb = const.tile([P, OT], F32, tag="wb_sb")
    nc.sync.dma_start(out=wb_sb, in_=wb_d.rearrange("(o p) -> p o", p=P))

    # ---- persistent tensors ---------------------------------------------
    zT = pers.tile([P, NU * FOUT], F32, tag="zT")
    augM = pers.tile([KA, MLOC], F32, tag="augM")
    augN = pers.tile([KA, N], F32, tag="augN")
    d2bc = pers.tile([P, N], F32, tag="d2bc")
    acc1 = pers.tile([P, NT * NCC], F32, tag="acc1")
    acc2 = pers.tile([P, NT * NCC], F32, tag="acc2")
    rs_old = pers.tile([P, NT * NCC], F32, tag="rs_old")
    deg1l = pers.tile([P, NT], F32, tag="deg1l")
    deg2l = pers.tile([P, NT], F32, tag="deg2l")
    d2p = pers.tile([P, NT], F32, tag="d2p")
    r17n = pers.tile([1, N], F32, tag="r17n")
    r16m = pers.tile([1, MLOC], F32, tag="r16m")
    out_sb = pers.tile([P, OT * MLOC], F32, tag="out_sb")

    # ======================= phase 0 =====================================
    with tc.tile_pool(name="xpool", bufs=1) as xpool, tc.tile_pool(
        name="ps0", bufs=3, space="PSUM"
    ) as ps0:
        x_sb = [xpool.tile([P, N], F32, name=f"x{k}", tag=f"x{k}") for k in range(KF)]
        xl_sb = [xpool.tile([P, MLOC], F32, name=f"xl{k}", tag=f"xl{k}") for k in range(KF)]
        for k in range(KF):
            nc.sync.dma_start(out=x_sb[k], in_=x_d[k * P : (k + 1) * P, :])
            nc.sync.dma_start(out=xl_sb[k], in_=xl_d[k * P : (k + 1) * P, :])

        rx = xpool.tile([RDIM, N], F32, tag="rx")
        rxl = xpool.tile([RDIM, MLOC], F32, tag="rxl")

        # R_w^T [FIN, RDIM] and W_w^T [FIN, FOUT] via PE transposes
        rw_nat = xpool.tile([RDIM, FIN], F32, tag="rw_nat")
        ww_nat = [xpool.tile([P, FIN], F32, name=f"ww_nat{o}", tag=f"ww_nat{o}") for o in range(OT)]
        nc.sync.dma_start(out=rw_nat, in_=rw_d[:, :])
        for o in range(OT):
            nc.sync.dma_start(out=ww_nat[o], in_=ww_d[o * P : (o + 1) * P, :])
        rwT = [xpool.tile([P, RDIM], F32, name=f"rwT{k}", tag=f"rwT{k}") for k in range(KF)]
        wwT = [xpool.tile([P, FOUT], F32, name=f"wwT{k}", tag=f"wwT{k}") for k in range(KF)]
        for k in range(KF):
            pt = ps0.tile([P, RDIM], F32, tag="ps0")
            nc.tensor.transpose(
                pt, rw_nat[:, k * P : (k + 1) * P], ident[:RDIM, :RDIM]
            )
            nc.scalar.copy(rwT[k], pt)
            for o in range(OT):
                pt2 = ps0.tile([P, P], F32, tag="ps0")
                nc.tensor.transpose(
                    pt2, ww_nat[o][:, k * P : (k + 1) * P], ident
                )
                nc.scalar.copy(wwT[k][:, o * P : (o + 1) * P], pt2)

        # rx = R_w @ x ; rxl = R_w @ x_local
        for c in range(NCC):
            pt = ps0.tile([RDIM, CW], F32, tag="ps0")
            for k in range(KF):
                nc.tensor.matmul(
                    pt,
                    rwT[k],
                    x_sb[k][:, c * CW : (c + 1) * CW],
                    start=(k == 0),
                    stop=(k == KF - 1),
                )
            nc.scalar.copy(rx[:, c * CW : (c + 1) * CW], pt)
        for h in range(NH):
            pt = ps0.tile([RDIM, HW_], F32, tag="ps0")
            for k in range(KF):
                nc.tensor.matmul(
                    pt,
                    rwT[k],
                    xl_sb[k][:, h * HW_ : (h + 1) * HW_],
                    start=(k == 0),
                    stop=(k == KF - 1),
                )
            nc.scalar.copy(rxl[:, h * HW_ : (h + 1) * HW_], pt)

        # sq rows staged at partition 0: r17n = -sq ; r16m = -sq_local
        for c in range(NCC):
            r2 = stream.tile([RDIM, CW], F32, tag="rx2")
            nc.vector.tensor_mul(
                r2, rx[:, c * CW : (c + 1) * CW], rx[:, c * CW : (c + 1) * CW]
            )
            pt = ps0.tile([1, CW], F32, tag="ps0")
            nc.tensor.matmul(pt, ones_r, r2, start=True, stop=True)
            nc.scalar.activation(
                r17n[:, c * CW : (c + 1) * CW], pt, AF.Copy, scale=-1.0
            )
        for h in range(NH):
            r2 = stream.tile([RDIM, HW_], F32, tag="rx2")
            nc.vector.tensor_mul(
                r2, rxl[:, h * HW_ : (h + 1) * HW_], rxl[:, h * HW_ : (h + 1) * HW_]
            )
            pt = ps0.tile([1, HW_], F32, tag="ps0")
            nc.tensor.matmul(pt, ones_r, r2, start=True, stop=True)
            nc.scalar.activation(
                r16m[:, h * HW_ : (h + 1) * HW_], pt, AF.Copy, scale=-1.0
            )
        # aug rx rows: augM rows0..15 = 2*rx_local ; augN rows0..15 = rx
        nc.vector.tensor_scalar_mul(augM[:RDIM, :], rxl, 2.0)
        nc.vector.tensor_copy(augN[:RDIM, :], rx)
        # special rows land at partitions 16/17 via DMA (engines can't
        # address non-32-aligned partition bases)
        nc.sync.dma_start(out=augN[RDIM : RDIM + 1, :], in_=ones_dram)
        nc.sync.dma_start(out=augN[RDIM + 1 : RDIM + 2, :], in_=r17n)
        nc.sync.dma_start(out=augM[RDIM : RDIM + 1, :], in_=r16m)
        nc.sync.dma_start(out=augM[RDIM + 1 : RDIM + 2, :], in_=ones_dram[:, :MLOC])

        # zT[u-chunk] = x_chunk^T @ W_w^T   (z = W_w @ x, transposed layout)
        for u in range(NU):
            pt = ps0.tile([P, FOUT], F32, tag="ps0")
            for k in range(KF):
                nc.tensor.matmul(
                    pt,
                    x_sb[k][:, u * P : (u + 1) * P],
                    wwT[k],
                    start=(k == 0),
                    stop=(k == KF - 1),
                )
            nc.vector.tensor_copy(zT[:, u * FOUT : (u + 1) * FOUT], pt)

        # old_A pass 1: row sums (streamed; values discarded)
        for t in range(NT):
            for c in range(NCC):
                oa = stream.tile([P, CW], F32, tag="oa")
                nc.sync.dma_start(
                    out=oa, in_=oa_d[t * P : (t + 1) * P, c * CW : (c + 1) * CW]
                )
                nc.vector.tensor_reduce(
                    rs_old[:, t * NCC + c : t * NCC + c + 1],
                    oa,
                    mybir.AxisListType.X,
                    ALU.add,
                )

    ps_mm = ctx.enter_context(tc.tile_pool(name="ps_mm", bufs=2, space="PSUM"))
    ps_tp = ctx.enter_context(tc.tile_pool(name="ps_tp", bufs=2, space="PSUM"))
    ps_xa = ctx.enter_context(tc.tile_pool(name="ps_xa", bufs=1, space="PSUM"))

    def gram_pass(acc):
        for t in range(NT):
            for c in range(NCC):
                th = ps_mm.tile([P, CW], F32, tag="theta")
                nc.tensor.matmul(
                    th,
                    augM[:, t * P : (t + 1) * P],
                    augN[:, c * CW : (c + 1) * CW],
                    start=True,
                    stop=True,
                )
                nc.scalar.activation(
                    th, th, AF.Exp,
                    accum_out=acc[:, t * NCC + c : t * NCC + c + 1],
                )

    # ============== phases 1 & 2: deg1 / deg2 + AllGathers ===============
    with tc.tile_pool(name="rows", bufs=1) as rows:
        # ---- phase 1
        gram_pass(acc1)
        for t in range(NT):
            nc.vector.tensor_reduce(
                deg1l[:, t : t + 1],
                acc1[:, t * NCC : (t + 1) * NCC],
                mybir.AxisListType.X,
                ALU.add,
            )
        nc.sync.dma_start(out=cc1i.rearrange("(t p) -> p t", p=P), in_=deg1l)
        nc.gpsimd.collective_compute(
            "AllGather", ALU.bypass, replica_groups=groups,
            ins=[cc1i[:]], outs=[cc1o[:]],
        )
        dgf = rows.tile([1, N], F32, tag="vrow_a")
        nc.sync.dma_start(out=dgf, in_=cc1o.rearrange("(a n) -> a n", a=1))
        nc.vector.tensor_scalar_max(dgf, dgf, 1.0)
        lnr = rows.tile([1, N], F32, tag="vrow_b")
        nc.scalar.activation(lnr, dgf, AF.Ln)
        nc.vector.tensor_scalar_mul(lnr, lnr, -0.5)
        nc.vector.tensor_add(r17n, r17n, lnr)
        nc.sync.dma_start(out=augN[RDIM + 1 : RDIM + 2, :], in_=r17n)
        dgl = rows.tile([1, MLOC], F32, tag="vrow_c")
        nc.sync.dma_start(out=dgl, in_=cc1i.rearrange("(a n) -> a n", a=1))
        nc.vector.tensor_scalar_max(dgl, dgl, 1.0)
        lnl = rows.tile([1, MLOC], F32, tag="vrow_d")
        nc.scalar.activation(lnl, dgl, AF.Ln)
        nc.vector.tensor_scalar_mul(lnl, lnl, -0.5)
        nc.vector.tensor_add(r16m, r16m, lnl)
        nc.sync.dma_start(out=augM[RDIM : RDIM + 1, :], in_=r16m)

        # ---- phase 2
        gram_pass(acc2)
        nc.vector.tensor_add(acc2, acc2, rs_old)
        for t in range(NT):
            nc.vector.tensor_reduce(
                deg2l[:, t : t + 1],
                acc2[:, t * NCC : (t + 1) * NCC],
                mybir.AxisListType.X,
                ALU.add,
            )
        # local d2 (partition layout) for the old_A row scaling
        nc.vector.tensor_scalar_max(d2p, deg2l, 1.0)
        nc.scalar.activation(d2p, d2p, AF.Sqrt)
        nc.vector.reciprocal(d2p, d2p)

        nc.sync.dma_start(out=cc2i.rearrange("(t p) -> p t", p=P), in_=deg2l)
        nc.gpsimd.collective_compute(
            "AllGather", ALU.bypass, replica_groups=groups,
            ins=[cc2i[:]], outs=[cc2o[:]],
        )
        dgf2 = rows.tile([1, N], F32, tag="vrow_a")
        nc.sync.dma_start(out=dgf2, in_=cc2o.rearrange("(a n) -> a n", a=1))
        nc.vector.tensor_scalar_max(dgf2, dgf2, 1.0)
        lnr2 = rows.tile([1, N], F32, tag="vrow_b")
        nc.scalar.activation(lnr2, dgf2, AF.Ln)
        nc.vector.tensor_scalar_mul(lnr2, lnr2, -0.5)
        nc.vector.tensor_add(r17n, r17n, lnr2)
        nc.sync.dma_start(out=augN[RDIM + 1 : RDIM + 2, :], in_=r17n)
        dgl2 = rows.tile([1, MLOC], F32, tag="vrow_c")
        nc.sync.dma_start(out=dgl2, in_=cc2i.rearrange("(a n) -> a n", a=1))
        nc.vector.tensor_scalar_max(dgl2, dgl2, 1.0)
        lnl2 = rows.tile([1, MLOC], F32, tag="vrow_d")
        nc.scalar.activation(lnl2, dgl2, AF.Ln)
        nc.vector.tensor_scalar_mul(lnl2, lnl2, -0.5)
        nc.vector.tensor_add(r16m, r16m, lnl2)
        nc.sync.dma_start(out=augM[RDIM : RDIM + 1, :], in_=r16m)

        # d2 row -> broadcast tile [128, N] by partition doubling
        d2r = rows.tile([1, N], F32, tag="vrow_e")
        nc.vector.tensor_scalar_max(d2r, dgf2, 1.0)
        nc.scalar.activation(d2r, d2r, AF.Sqrt)
        nc.vector.reciprocal(d2r, d2r)
        for c in range(NCC):
            ptb = ps_mm.tile([P, CW], F32, tag="theta")
            nc.tensor.matmul(
                ptb, ones_c, d2r[:, c * CW : (c + 1) * CW], start=True, stop=True
            )
            nc.vector.tensor_copy(d2bc[:, c * CW : (c + 1) * CW], ptb)

    # ======================= phase 3: A2 + out ===========================
    with tc.tile_pool(name="strips", bufs=2 * NS) as strip_pool:
        xa_ps = [ps_xa.tile([P, MLOC], F32, name=f"xa{ot}", tag=f"xa{ot}") for ot in range(OT)]
        for c in range(NCC):
            strips = [
                strip_pool.tile([P, MLOC], F32, name=f"strip_{c}_{i}", tag="strip")
                for i in range(NS)
            ]
            for t in range(NT):
                th = ps_mm.tile([P, CW], F32, tag="theta")
                nc.tensor.matmul(
                    th,
                    augM[:, t * P : (t + 1) * P],
                    augN[:, c * CW : (c + 1) * CW],
                    start=True,
                    stop=True,
                )
                t1 = stream.tile([P, CW], F32, tag="t1")
                nc.scalar.activation(t1, th, AF.Exp)
                oa = stream.tile([P, CW], F32, tag="oa")
                nc.sync.dma_start(
                    out=oa, in_=oa_d[t * P : (t + 1) * P, c * CW : (c + 1) * CW]
                )
                v = stream.tile([P, CW], F32, tag="v")
                nc.vector.scalar_tensor_tensor(
                    out=v,
                    in0=oa,
                    scalar=d2p[:, t : t + 1],
                    in1=d2bc[:, c * CW : (c + 1) * CW],
                    op0=ALU.mult,
                    op1=ALU.mult,
                )
                a2 = stream.tile([P, CW], F32, tag="a2")
                nc.gpsimd.tensor_tensor(out=a2, in0=t1, in1=v, op=ALU.add)
                nc.sync.dma_start(
                    out=aout_d[t * P : (t + 1) * P, c * CW : (c + 1) * CW],
                    in_=a2,
                )
                # transpose A2 tile into strips for the xa matmul
                for sub in range(NS):
                    tp = ps_tp.tile([P, P], F32, tag="tp")
                    nc.tensor.transpose(tp, a2[:, sub * P : (sub + 1) * P], ident)
                    if (t + sub) % 2 == 0:
                        nc.scalar.copy(strips[sub][:, t * P : (t + 1) * P], tp)
                    else:
                        nc.vector.tensor_copy(
                            strips[sub][:, t * P : (t + 1) * P], tp
                        )
            # xa accumulation: out[o, m] += (z^T chunk)^T @ (A2^T chunk)
            for sub in range(NS):
                u = c * NS + sub
                for ot in range(OT):
                    for h in range(NH):
                        nc.tensor.matmul(
                            xa_ps[ot][:, h * HW_ : (h + 1) * HW_],
                            zT[:, u * FOUT + ot * P : u * FOUT + (ot + 1) * P],
                            strips[sub][:, h * HW_ : (h + 1) * HW_],
                            start=(c == 0 and sub == 0),
                            stop=(c == NCC - 1 and sub == NS - 1),
                        )
        for ot in range(OT):
            nc.scalar.activation(
                out_sb[:, ot * MLOC : (ot + 1) * MLOC],
                xa_ps[ot],
                AF.Identity,
                bias=wb_sb[:, ot : ot + 1],
            )
            nc.sync.dma_start(
                out=out_d[ot * P : (ot + 1) * P, :],
                in_=out_sb[:, ot * MLOC : (ot + 1) * MLOC],
            )


def build_nc(cfg=CFG):
    B, FIN, FOUT, N, RDIM = (
        cfg["B"], cfg["FIN"], cfg["FOUT"], cfg["N"], cfg["RDIM"],
    )
    n_cores = cfg["n_cores"]
    GROUP = n_cores // B
    MLOC = N // GROUP
    nc = bacc.Bacc(
        "TRN2",
        target_bir_lowering=False,
        debug=False,
        enable_asserts=False,
        num_devices=n_cores,
    )
    io = {
        "x": nc.dram_tensor("x", [FIN, N], F32, kind="ExternalInput").ap(),
        "x_local": nc.dram_tensor("x_local", [FIN, MLOC], F32, kind="ExternalInput").ap(),
        "old_A": nc.dram_tensor("old_A", [MLOC, N], F32, kind="ExternalInput").ap(),
        "R_w": nc.dram_tensor("R_w", [RDIM, FIN], F32, kind="ExternalInput").ap(),
        "W_w": nc.dram_tensor("W_w", [FOUT, FIN], F32, kind="ExternalInput").ap(),
        "W_b": nc.dram_tensor("W_b", [FOUT], F32, kind="ExternalInput").ap(),
        "A_out": nc.dram_tensor("A_out", [MLOC, N], F32, kind="ExternalOutput").ap(),
        "out_f": nc.dram_tensor("out_f", [FOUT, MLOC], F32, kind="ExternalOutput").ap(),
        "cc1i": nc.dram_tensor("cc1i", [MLOC], F32, kind="Internal").ap(),
        "cc1o": nc.dram_tensor("cc1o", [N], F32, kind="Internal").ap(),
        "cc2i": nc.dram_tensor("cc2i", [MLOC], F32, kind="Internal").ap(),
        "cc2o": nc.dram_tensor("cc2o", [N], F32, kind="Internal").ap(),
    }
    with tile.TileContext(nc) as tc:
        with ExitStack() as ctx:
            build_kernel_body(ctx, tc, io, cfg)
    nc.compile()
    return nc


def make_in_maps(x, old_A, R_w, W_w, W_b, cfg=CFG):
    B, N = cfg["B"], cfg["N"]
    n_cores = cfg["n_cores"]
    GROUP = n_cores // B
    MLOC = N // GROUP
    in_maps = []
    for core in range(n_cores):
        b = core // GROUP
        p = core % GROUP
        r0, r1 = p * MLOC, (p + 1) * MLOC
        in_maps.append(
            {
                "x": np.ascontiguousarray(x[b]),
                "x_local": np.ascontiguousarray(x[b][:, r0:r1]),
                "old_A": np.ascontiguousarray(old_A[b][r0:r1, :]),
                "R_w": np.ascontiguousarray(R_w),
                "W_w": np.ascontiguousarray(W_w),
                "W_b": np.ascontiguousarray(W_b),
            }
        )
    return in_maps


_NC_CACHE = {}


def kernel(x, old_A, R_w, W_w, W_b):
    from concourse import bass_utils

    cfg = CFG
    x = np.asarray(x, dtype=np.float32)
    old_A = np.asarray(old_A, dtype=np.float32)
    R_w = np.asarray(R_w, dtype=np.float32)
    W_w = np.asarray(W_w, dtype=np.float32)
    W_b = np.asarray(W_b, dtype=np.float32)

    if "nc" not in _NC_CACHE:
        _NC_CACHE["nc"] = build_nc(cfg)
    nc = _NC_CACHE["nc"]

    in_maps = make_in_maps(x, old_A, R_w, W_w, W_b, cfg)
    res = bass_utils.run_bass_kernel_spmd(
        nc, in_maps, core_ids=list(range(cfg["n_cores"]))
    )
    results = res.results

    B, FIN, FOUT, N = cfg["B"], cfg["FIN"], cfg["FOUT"], cfg["N"]
    n_cores = cfg["n_cores"]
    GROUP = n_cores // B
    MLOC = N // GROUP
    out = np.empty((B, FOUT, N), dtype=np.float32)
    A = np.empty((B, N, N), dtype=np.float32)
    for core in range(n_cores):
        b = core // GROUP
        p = core % GROUP
        r0, r1 = p * MLOC, (p + 1) * MLOC
        A[b, r0:r1, :] = results[core]["A_out"]
        out[b, :, r0:r1] = results[core]["out_f"]
    return (out, A)
